# revision 11
# baseline (speedup 1.0000x reference)
"""Trainium2 Bass kernel for a dense transformer block (pre-LN, MHA + MLP).

Sharding: 8 cores. Batch (B=2) is split across two 4-core groups; within a
group each core computes the full LayerNorm/K/V for its batch (2048 tokens)
but only its own 512 query rows through attention, proj, and the MLP.
No collectives: the host rolls each core's batch so its query rows are
tokens [0:512], and the host gathers the 8 x [512, 1024] results.

fp8 (e4m3) DoubleRow matmuls everywhere except the S=q@k matmul (bf16) and
the tiny znorm/shift matmuls.  DoubleRow contracts two 128-K tiles per
instruction at 0.5 cycles/output-column (4x bf16 FLOP rate).  Weights are
scaled x32 into fp8 range host-side; descales fold into existing psum-drain
ops (tensor_scalar / activation scale), so they are free.  W1/W2 are split
into fp8 hi+lo pairs (W ~ hi+lo with ~0.3% error) because single-fp8 MLP
weights push the output past the 2e-2 tolerance; attention tolerates plain
fp8 (softmax jitter averages out over 2048 keys).

gamma/beta of both LayerNorms and the q-scale are folded into the weight
matrices host-side (LN(x) @ W == xhat @ (gamma*W) + beta@W).
"""

import numpy as np
import ml_dtypes

import concourse.bass as bass
import concourse.tile as tile
from concourse import bacc, mybir
from concourse.bass_utils import run_bass_kernel_spmd
from concourse.masks import make_identity

F32 = mybir.dt.float32
BF16 = mybir.dt.bfloat16
F8 = mybir.dt.float8e4
AF = mybir.ActivationFunctionType
ALU = mybir.AluOpType
DR = mybir.MatmulPerfMode.DoubleRow

B, N, C, F, H, D = 2, 2048, 1024, 4096, 16, 64
QR = 512            # query rows per core
CH = C // 128       # 8 C-chunks
FT = F // 128       # 32 F-tiles
NT = N // 128       # 16 token tiles
EPS = 1e-6
NCORES = 8
WS = 32.0           # fp8 weight scale
RS = float(1.0 / WS)

_NC_CACHE = {}


def _ln_rstd(nc, stat, mv, eps_sb):
    sd = stat.tile([128, 1], F32, tag="sd", name="sd")
    nc.scalar.activation(out=sd, in_=mv[:, 1:2], func=AF.Sqrt, bias=eps_sb, scale=1.0)
    rstd = stat.tile([128, 1], F32, tag="rstd", name="rstd")
    nc.vector.reciprocal(out=rstd, in_=sd)
    return rstd


def _build_nc(zero_bias_r=False):
    nc = bacc.Bacc("TRN2", target_bir_lowering=False, debug=False,
                   enable_asserts=False, num_devices=NCORES)

    xb = nc.dram_tensor("xb", [N, C], BF16, kind="ExternalInput")
    xr = nc.dram_tensor("xr", [QR, C], F32, kind="ExternalInput")
    wq = nc.dram_tensor("wq", [C, C], F8, kind="ExternalInput")
    wk = nc.dram_tensor("wk", [C, C], F8, kind="ExternalInput")
    wv = nc.dram_tensor("wv", [C, C], F8, kind="ExternalInput")
    wp = nc.dram_tensor("wp", [C, C], F8, kind="ExternalInput")
    w1t = nc.dram_tensor("w1t", [2, FT, 128, CH, 128], F8, kind="ExternalInput")
    w2 = nc.dram_tensor("w2", [2, F, C], F8, kind="ExternalInput")
    biasT = nc.dram_tensor("biasT", [128, CH + CH + FT], F32, kind="ExternalInput")
    biasR = nc.dram_tensor("biasR", [3, C], BF16, kind="ExternalInput")
    out = nc.dram_tensor("out", [QR, C], F32, kind="ExternalOutput")

    with tile.TileContext(nc) as tc, \
         tc.tile_pool(name="consts", bufs=1) as consts, \
         tc.tile_pool(name="wc", bufs=8) as wcp, \
         tc.tile_pool(name="w1p", bufs=6) as w1p, \
         tc.tile_pool(name="big", bufs=1) as big, \
         tc.tile_pool(name="xin", bufs=3) as xin, \
         tc.tile_pool(name="xhp", bufs=3) as xhp, \
         tc.tile_pool(name="stat", bufs=4) as stat, \
         tc.tile_pool(name="expp", bufs=10) as expp, \
         tc.tile_pool(name="outp", bufs=2) as outp, \
         tc.tile_pool(name="zp", bufs=2) as zp, \
         tc.tile_pool(name="ps", bufs=2, space="PSUM") as ps:

        # ---- constants ----
        ident = consts.tile([128, 128], BF16, name="ident")
        make_identity(nc, ident)
        onesP = consts.tile([128, 128], BF16, name="onesP")
        nc.vector.memset(onesP, 1.0)
        shift_sb = consts.tile([128, 128], BF16, name="shift_sb")
        nc.gpsimd.memset(shift_sb, 0.0)
        nc.gpsimd.affine_select(out=shift_sb, in_=shift_sb,
                                compare_op=ALU.not_equal, fill=1.0, base=64,
                                pattern=[[-1, 128]], channel_multiplier=1)
        ones1 = onesP[0:1, :]
        onesF8 = consts.tile([128, 2, 64], F8, name="onesF8")
        nc.vector.memset(onesF8, 1.0)
        eps_sb = consts.tile([128, 1], F32, name="eps_sb")
        nc.vector.memset(eps_sb, EPS)
        biasT_sb = consts.tile([128, CH + CH + FT], F32, name="biasT_sb")
        nc.sync.dma_start(out=biasT_sb, in_=biasT.ap())
        bq_sb = biasT_sb[:, 0:CH]
        bk_sb = biasT_sb[:, CH:2 * CH]
        b1_sb = biasT_sb[:, 2 * CH:2 * CH + FT]
        if not zero_bias_r:
            biasRow = consts.tile([65, C], BF16, name="biasRow")
            nc.sync.dma_start(out=biasRow[::32, :], in_=biasR.ap())
            bv_sb = biasRow[0:1, :]
            bp_sb = biasRow[32:33, :]
            b2_sb = biasRow[64:65, :]
        else:
            bv_sb = bp_sb = b2_sb = None

        # ---- LN1 + transpose -> xhatT [128, CH, N] (fp8) ----
        # transposes run in bf16 (fp8 PE-transpose needs strided outputs);
        # 4 chunk-transposes pack into one [128,512] bf16 psum tile and one
        # fp8-converting drain.
        xhatT = big.tile([128, CH, N], F8, tag="xhatT_hT", name="xhatT")

        def emit_ln1(t2):
            xt2 = xin.tile([128, 2, C], BF16, tag="xt", name="xt2")
            nc.sync.dma_start(
                out=xt2, in_=xb.ap()[t2 * 256:(t2 + 1) * 256, :]
                .rearrange("(i p) n -> p i n", p=128))
            for i in range(2):
                t = t2 * 2 + i
                xt = xt2[:, i, :]
                st = stat.tile([128, 2, 6], F32, tag="bns", name="st")
                nc.vector.bn_stats(out=st[:, 0, :], in_=xt[:, 0:512])
                nc.vector.bn_stats(out=st[:, 1, :], in_=xt[:, 512:1024])
                mv = stat.tile([128, 2], F32, tag="mv", name="mv")
                nc.vector.bn_aggr(out=mv, in_=st)
                rstd = _ln_rstd(nc, stat, mv, eps_sb)
                xh = xhp.tile([128, C], BF16, tag="xhat", name="xh", bufs=2)
                # normalize on Pool (sbuf->sbuf; keeps DVE for psum drains)
                nc.gpsimd.tensor_scalar(out=xh, in0=xt, scalar1=mv[:, 0:1],
                                        scalar2=rstd,
                                        op0=ALU.subtract, op1=ALU.mult)
                for half in range(2):
                    pst = ps.tile([128, 4, 128], BF16, tag="mm512", name="pst")
                    for j in range(4):
                        c = half * 4 + j
                        nc.tensor.transpose(pst[:, j, :],
                                            xh[:, c * 128:(c + 1) * 128], ident)
                    nc.vector.tensor_copy(
                        out=xhatT[:, half * 4:half * 4 + 4, t * 128:(t + 1) * 128],
                        in_=pst)

        for t2 in range(2):
            emit_ln1(t2)

        # ---- QKV + attention, two 8-head groups ----
        yT = big.tile([128, H // 2, QR], F8, tag="yT_xh2L", name="yT")

        def dma_w(w, g, name):
            wt = wcp.tile([128, CH, 512], F8, tag="wh", name=name)
            nc.sync.dma_start(
                out=wt, in_=w.ap()[:, g * 512:(g + 1) * 512]
                .rearrange("(c p) n -> p c n", p=128))
            return wt

        def emit_qT(g, wq_h, qT_g, js=None):
            for j in (range(4) if js is None else js):
                psq = ps.tile([128, 512], F32, tag="mm512", name="psq")
                for qh in range(2):
                    for cp in range(4):
                        nc.tensor.matmul(
                            psq[:, qh * 256:(qh + 1) * 256],
                            lhsT=wq_h[:, 2 * cp:2 * cp + 2, j * 128:(j + 1) * 128],
                            rhs=xhatT[:, 2 * cp:2 * cp + 2, qh * 256:(qh + 1) * 256],
                            start=(cp == 0), stop=(cp == 3), perf_mode=DR)
                nc.vector.tensor_scalar(
                    out=qT_g[:, j, :], in0=psq, scalar1=RS,
                    scalar2=bq_sb[:, 4 * g + j:4 * g + j + 1],
                    op0=ALU.mult, op1=ALU.add)

        def emit_kT_chunk(g, wk_h, kT_g, j, sl):
            psk = ps.tile([128, 512], F32, tag="mm512", name="psk")
            for qh in range(2):
                off = sl * 512 + qh * 256
                for cp in range(4):
                    nc.tensor.matmul(
                        psk[:, qh * 256:(qh + 1) * 256],
                        lhsT=wk_h[:, 2 * cp:2 * cp + 2, j * 128:(j + 1) * 128],
                        rhs=xhatT[:, 2 * cp:2 * cp + 2, off:off + 256],
                        start=(cp == 0), stop=(cp == 3), perf_mode=DR)
            nc.vector.tensor_scalar(
                out=kT_g[:, j, sl * 512:(sl + 1) * 512], in0=psk, scalar1=RS,
                scalar2=bk_sb[:, 4 * g + j:4 * g + j + 1],
                op0=ALU.mult, op1=ALU.add)

        def emit_v(g, wv_h, vaug, tts=None):
            # vaug holds 32*v in fp8
            for tt in (range(NT) if tts is None else tts):
                psv = ps.tile([128, 512], F32, tag="mm512", name="psv")
                for vh in range(2):
                    for cp in range(4):
                        nc.tensor.matmul(
                            psv[:, vh * 256:(vh + 1) * 256],
                            lhsT=xhatT[:, 2 * cp:2 * cp + 2, tt * 128:(tt + 1) * 128],
                            rhs=wv_h[:, 2 * cp:2 * cp + 2, vh * 256:(vh + 1) * 256],
                            start=(cp == 0),
                            stop=(zero_bias_r and cp == 3), perf_mode=DR)
                if not zero_bias_r:
                    nc.tensor.matmul(psv, lhsT=ones1,
                                     rhs=bv_sb[:, g * 512:(g + 1) * 512],
                                     start=False, stop=True,
                                     skip_group_check=True)
                nc.vector.tensor_copy(out=vaug[:, tt, :, :],
                                      in_=psv.rearrange("p (h d) -> p h d", h=8))

        def emit_znorm(g, hh, psYZ):
            # psYZ[:,0,:] = 32*sum(e*v); psYZ[:,1,:] = Z (all 64 rows)
            rz = zp.tile([64, 512], BF16, tag="rz", name="rz")
            with nc.allow_low_precision("1/Z in bf16"):
                nc.vector.reciprocal(out=rz, in_=psYZ[:, 1, :])
            pc = (8 * g + hh) // 2
            if hh % 2 == 0:
                nc.vector.tensor_mul(out=yT[0:64, pc, :],
                                     in0=psYZ[:, 0, :], in1=rz)
            else:
                yn = zp.tile([64, 512], BF16, tag="yn", name="yn")
                nc.vector.tensor_mul(out=yn, in0=psYZ[:, 0, :], in1=rz)
                psSh = ps.tile([128, 512], F32, tag="mm512", name="psSh")
                nc.tensor.matmul(psSh, lhsT=shift_sb[0:64, :], rhs=yn,
                                 start=True, stop=True)
                nc.vector.tensor_copy(out=yT[64:128, pc, :],
                                      in_=psSh[64:128, :])

        def emit_attn_head(g, hh, kT_g, qT_g, vaug, filler, prev_znorm):
            jj = hh // 2
            poff = (hh % 2) * 64
            psYZ = ps.tile([64, 2, 512], F32, tag="y", name="psYZ", bufs=1)

            # S for all 16 key tiles (8 blocks of 2), exp -> fp8 expS tiles
            exps = []
            for tp in range(8):
                psS = ps.tile([128, 1024], F32, tag="s1024", name="psS")
                for half in range(2):
                    tt = 2 * tp + half
                    nc.tensor.matmul(
                        psS[:, half * 512:(half + 1) * 512],
                        lhsT=kT_g[poff:poff + 64, jj, tt * 128:(tt + 1) * 128],
                        rhs=qT_g[poff:poff + 64, jj, :],
                        start=True, stop=True)
                expS = expp.tile([128, 2, 512], F8, tag="expS", name="expS")
                nc.scalar.activation(out=expS, in_=psS, func=AF.Exp)
                exps.append(expS)
                if tp == 0:
                    if prev_znorm is not None:
                        prev_znorm()
                    if filler is not None:
                        filler()

            # Y and Z accumulation, one 256-query half at a time so each
            # psum accumulation group is sequential in its region.
            for sel in range(2):            # 0: y = 32*e@v, 1: Z = e@1
                for qh in range(2):
                    qsl = slice(qh * 256, (qh + 1) * 256)
                    for tp in range(8):
                        nc.tensor.matmul(
                            psYZ[:, sel, qsl],
                            lhsT=(vaug[:, 2 * tp:2 * tp + 2, hh, :] if sel == 0
                                  else onesF8),
                            rhs=exps[tp][:, :, qsl],
                            start=(tp == 0), stop=(tp == 7), perf_mode=DR)
            return lambda: emit_znorm(g, hh, psYZ)

        # group 0 QKV, interleaved with the remaining LN1 slices
        wq_h0 = dma_w(wq, 0, "wq_h0")
        wk_h0 = dma_w(wk, 0, "wk_h0")
        wv_h0 = dma_w(wv, 0, "wv_h0")
        qT_g0 = big.tile([128, 4, QR], BF16, tag="qT_xh2T", name="qT_g0", bufs=2)
        emit_qT(0, wq_h0, qT_g0)
        kT_g0 = big.tile([128, 4, N], BF16, tag="kT_xr", name="kT_g0", bufs=2)
        vaug0 = big.tile([128, NT, 8, 64], F8, tag="vaug_x2", name="vaug0")
        for sl in range(4):
            for j in range(4):
                emit_kT_chunk(0, wk_h0, kT_g0, j, sl)
            emit_v(0, wv_h0, vaug0, range(4 * sl, 4 * sl + 4))
            if sl < 3:
                emit_ln1(2 * (sl + 1))
                emit_ln1(2 * (sl + 1) + 1)

        # group-0 attention: filler worklist = all of qT(g1) plus kT(g1)
        # chunks for head-pairs 0-1; pair-2/3 kT chunks are deferred into
        # group-1's attention.
        wk_h1 = dma_w(wk, 1, "wk_h1")
        wq_h1 = dma_w(wq, 1, "wq_h1")
        kT_g1 = big.tile([128, 4, N], BF16, tag="kT_xr", name="kT_g1", bufs=2)
        qT_g1 = big.tile([128, 4, QR], BF16, tag="qT_xh2T", name="qT_g1", bufs=2)

        units = [lambda j=j: emit_qT(1, wq_h1, qT_g1, [j]) for j in range(4)]
        units += [lambda j=j, sl=sl: emit_kT_chunk(1, wk_h1, kT_g1, j, sl)
                  for j in range(2) for sl in range(4)]

        def g0_filler(hh):
            lo = (hh * len(units)) // 8
            hi = ((hh + 1) * len(units)) // 8

            def fill():
                for u in range(lo, hi):
                    units[u]()
            return fill

        zn = None
        for hh in range(8):
            zn = emit_attn_head(0, hh, kT_g0, qT_g0, vaug0, g0_filler(hh), zn)
        zn()

        # group 1 remaining QKV (v only)
        wv_h1 = dma_w(wv, 1, "wv_h1")
        vaug1 = big.tile([128, NT, 8, 64], F8, tag="vaug_x2", name="vaug1")
        emit_v(1, wv_h1, vaug1)

        # group-1 attention with first-half proj as PE filler.
        # proj: psp = yT(32y) @ wp(32Wp) = 1024*(y@Wp); descale 2^-10 folds
        # into the drain.
        wp_ts = [dma_w(wp, 0, "wp_h0"), dma_w(wp, 1, "wp_h1")]
        proj_part = xhp.tile([128, QR // 128, C], BF16, tag="ppart", name="proj_part",
                             bufs=1)
        PS2 = float(RS * RS)

        def proj_filler(hh):
            os_, qt = hh // 4, hh % 4

            def fill():
                if hh < 4:
                    emit_kT_chunk(1, wk_h1, kT_g1, 2, hh)
                elif hh < 6:
                    emit_kT_chunk(1, wk_h1, kT_g1, 3, 2 * (hh - 4))
                    emit_kT_chunk(1, wk_h1, kT_g1, 3, 2 * (hh - 4) + 1)
                psp = ps.tile([128, 512], F32, tag="mm512", name="psp")
                for ph in range(2):
                    for pp in range(2):
                        nc.tensor.matmul(
                            psp[:, ph * 256:(ph + 1) * 256],
                            lhsT=yT[:, 2 * pp:2 * pp + 2, qt * 128:(qt + 1) * 128],
                            rhs=wp_ts[os_][:, 2 * pp:2 * pp + 2,
                                           ph * 256:(ph + 1) * 256],
                            start=(pp == 0), stop=(pp == 1), perf_mode=DR)
                nc.vector.tensor_scalar(
                    out=proj_part[:, qt, os_ * 512:(os_ + 1) * 512], in0=psp,
                    scalar1=PS2, scalar2=None, op0=ALU.mult)
            return fill

        zn = None
        for hh in range(8):
            zn = emit_attn_head(1, hh, kT_g1, qT_g1, vaug1, proj_filler(hh), zn)
        zn()

        # ---- proj second half + partial + residual -> x2 ----
        xr_sb = big.tile([128, QR // 128, C], F32, tag="kT_xr", name="xr_sb", bufs=2)
        nc.sync.dma_start(out=xr_sb, in_=xr.ap().rearrange("(q p) c -> p q c", p=128))
        x2 = big.tile([128, QR // 128, C], F32, tag="vaug_x2", name="x2")
        for os_ in range(2):
            for qt in range(QR // 128):
                psp = ps.tile([128, 512], F32, tag="mm512", name="psp2")
                for ph in range(2):
                    for pp in range(2, 4):
                        nc.tensor.matmul(
                            psp[:, ph * 256:(ph + 1) * 256],
                            lhsT=yT[:, 2 * pp:2 * pp + 2, qt * 128:(qt + 1) * 128],
                            rhs=wp_ts[os_][:, 2 * pp:2 * pp + 2,
                                           ph * 256:(ph + 1) * 256],
                            start=(pp == 2),
                            stop=(zero_bias_r and pp == 3), perf_mode=DR)
                if not zero_bias_r:
                    nc.tensor.matmul(psp, lhsT=onesP[32:33, :],
                                     rhs=bp_sb[:, os_ * 512:(os_ + 1) * 512],
                                     start=False, stop=True,
                                     skip_group_check=True)
                sl = slice(os_ * 512, (os_ + 1) * 512)
                nc.vector.scalar_tensor_tensor(
                    out=x2[:, qt, sl], in0=psp, scalar=PS2,
                    in1=proj_part[:, qt, sl], op0=ALU.mult, op1=ALU.add)
                nc.vector.tensor_add(out=x2[:, qt, sl], in0=x2[:, qt, sl],
                                     in1=xr_sb[:, qt, sl])

        # ---- LN2 + transpose -> xhat2T hi/lo [128, CH, QR] (fp8) ----
        # xh2T is the fp8 rounding of xhat2; xh2L holds the rounding residual
        # (also fp8, unscaled: residuals live in fp8's subnormal range), so
        # MLP1 can contract (hi + lo) @ W1_hi for ~0.1% effective input
        # precision.
        xh2T = big.tile([128, CH, QR], F8, tag="qT_xh2T", name="xh2T", bufs=2)
        xh2L = big.tile([128, CH, QR], F8, tag="yT_xh2L", name="xh2L")
        for qt in range(QR // 128):
            st2 = stat.tile([128, 2, 6], F32, tag="bns", name="st2")
            nc.vector.bn_stats(out=st2[:, 0, :], in_=x2[:, qt, 0:512])
            nc.vector.bn_stats(out=st2[:, 1, :], in_=x2[:, qt, 512:1024])
            mv2 = stat.tile([128, 2], F32, tag="mv", name="mv2")
            nc.vector.bn_aggr(out=mv2, in_=st2)
            rstd2 = _ln_rstd(nc, stat, mv2, eps_sb)
            xh2 = xhp.tile([128, C], BF16, tag="xhat", name="xh2", bufs=2)
            nc.gpsimd.tensor_scalar(out=xh2, in0=x2[:, qt, :], scalar1=mv2[:, 0:1],
                                    scalar2=rstd2, op0=ALU.subtract, op1=ALU.mult)
            for half in range(2):
                pst2 = ps.tile([128, 4, 128], BF16, tag="mm512", name="pst2")
                for j in range(4):
                    c = half * 4 + j
                    nc.tensor.transpose(pst2[:, j, :],
                                        xh2[:, c * 128:(c + 1) * 128], ident)
                hsl = slice(half * 4, half * 4 + 4)
                qsl = slice(qt * 128, (qt + 1) * 128)
                nc.vector.tensor_copy(out=xh2T[:, hsl, qsl], in_=pst2)
                nc.vector.tensor_sub(out=xh2L[:, hsl, qsl], in0=pst2,
                                     in1=xh2T[:, hsl, qsl])

        # ---- MLP1: hT [128, FT, QR] fp8, fused exact Gelu + bias ----
        # psh = 32*(xn2 @ W1'): gelu(psh * 2^-5 + b1)
        hT = big.tile([128, FT, QR], F8, tag="xhatT_hT", name="hT")
        for ft in range(FT):
            w1hi = w1p.tile([128, CH, 128], F8, tag="w1", name="w1hi")
            nc.sync.dma_start(out=w1hi, in_=w1t.ap()[0, ft])
            w1lo = w1p.tile([128, CH, 128], F8, tag="w1", name="w1lo")
            nc.sync.dma_start(out=w1lo, in_=w1t.ap()[1, ft])
            psh = ps.tile([128, 512], F32, tag="mm512", name="psh")
            for qh in range(2):
                qsl = slice(qh * 256, (qh + 1) * 256)
                for cp in range(4):
                    nc.tensor.matmul(
                        psh[:, qsl], lhsT=w1hi[:, 2 * cp:2 * cp + 2, :],
                        rhs=xh2T[:, 2 * cp:2 * cp + 2, qsl],
                        start=(cp == 0), stop=False, perf_mode=DR)
                for cp in range(4):
                    nc.tensor.matmul(
                        psh[:, qsl], lhsT=w1lo[:, 2 * cp:2 * cp + 2, :],
                        rhs=xh2T[:, 2 * cp:2 * cp + 2, qsl],
                        start=False, stop=False, perf_mode=DR)
                for cp in range(4):
                    nc.tensor.matmul(
                        psh[:, qsl], lhsT=w1hi[:, 2 * cp:2 * cp + 2, :],
                        rhs=xh2L[:, 2 * cp:2 * cp + 2, qsl],
                        start=False, stop=(cp == 3), perf_mode=DR)
            nc.scalar.activation(out=hT[:, ft, :], in_=psh, func=AF.Gelu,
                                 bias=b1_sb[:, ft:ft + 1], scale=RS)

        # ---- MLP2 + residual -> out;  pso = 32*(h @ W2') ----
        for os_ in range(2):
            w2_hi, w2_lo = [], []
            for i in range(4):
                for dst, hl in ((w2_hi, 0), (w2_lo, 1)):
                    w2t = wcp.tile([128, CH, 512], F8, tag="wh", name="w2t")
                    nc.sync.dma_start(
                        out=w2t, in_=w2.ap()[hl, i * 1024:(i + 1) * 1024,
                                             os_ * 512:(os_ + 1) * 512]
                        .rearrange("(c p) n -> p c n", p=128))
                    dst.append(w2t)
            for qt in range(QR // 128):
                pso = ps.tile([128, 512], F32, tag="mm512", name="pso")
                for ph in range(2):
                    for src, first, last in ((w2_hi, True, False),
                                             (w2_lo, False, True)):
                        for fp in range(16):
                            nc.tensor.matmul(
                                pso[:, ph * 256:(ph + 1) * 256],
                                lhsT=hT[:, 2 * fp:2 * fp + 2,
                                        qt * 128:(qt + 1) * 128],
                                rhs=src[fp // 4][:, 2 * (fp % 4):2 * (fp % 4) + 2,
                                                 ph * 256:(ph + 1) * 256],
                                start=(first and fp == 0),
                                stop=(last and fp == 15 and zero_bias_r),
                                perf_mode=DR)
                if not zero_bias_r:
                    nc.tensor.matmul(pso, lhsT=onesP[64:65, :],
                                     rhs=b2_sb[:, os_ * 512:(os_ + 1) * 512],
                                     start=False, stop=True,
                                     skip_group_check=True)
                ot = outp.tile([128, 512], F32, tag="ot", name="ot")
                nc.vector.scalar_tensor_tensor(
                    out=ot, in0=pso, scalar=RS,
                    in1=x2[:, qt, os_ * 512:(os_ + 1) * 512],
                    op0=ALU.mult, op1=ALU.add)
                nc.sync.dma_start(out=out.ap()[qt * 128:(qt + 1) * 128,
                                               os_ * 512:(os_ + 1) * 512], in_=ot)

    nc.finalize()
    return nc


def _get_nc(zero_bias_r=False):
    key = ("nc", zero_bias_r)
    if key not in _NC_CACHE:
        _NC_CACHE[key] = _build_nc(zero_bias_r)
    return _NC_CACHE[key]


def kernel(x, Wq, Wk, Wv, Wp, bp, W1, b1, W2, b2, gamma1, beta1, gamma2, beta2):
    bf = ml_dtypes.bfloat16
    f8 = ml_dtypes.float8_e4m3
    x = np.asarray(x, np.float32)
    Wq = np.asarray(Wq, np.float32)
    Wk = np.asarray(Wk, np.float32)
    Wv = np.asarray(Wv, np.float32)
    Wp = np.asarray(Wp, np.float32)
    bp = np.asarray(bp, np.float32)
    W1 = np.asarray(W1, np.float32)
    b1 = np.asarray(b1, np.float32)
    W2 = np.asarray(W2, np.float32)
    b2 = np.asarray(b2, np.float32)
    gamma1 = np.asarray(gamma1, np.float32)
    beta1 = np.asarray(beta1, np.float32)
    gamma2 = np.asarray(gamma2, np.float32)
    beta2 = np.asarray(beta2, np.float32)

    scale = np.float32(D ** -0.5)
    wq_f = ((gamma1[:, None] * Wq) * (scale * WS)).astype(f8)
    bq_f = (beta1 @ Wq) * scale
    wk_f = ((gamma1[:, None] * Wk) * WS).astype(f8)
    bk_f = beta1 @ Wk
    wv_f = ((gamma1[:, None] * Wv) * WS).astype(f8)
    bv_f = beta1 @ Wv
    w1_f = (gamma2[:, None] * W1) * WS
    b1_f = b1 + beta2 @ W1

    def hilo(Ws):
        hi = Ws.astype(f8)
        lo = (Ws - hi.astype(np.float32)).astype(f8)
        return hi, lo

    w1_hi, w1_lo = hilo(w1_f)
    w1_tiled = np.stack([
        np.ascontiguousarray(
            w.reshape(CH, 128, FT, 128).transpose(2, 1, 0, 3))
        for w in (w1_hi, w1_lo)])
    w2_hi, w2_lo = hilo(W2 * WS)
    w2_st = np.stack([w2_hi, w2_lo])
    biasT = np.ascontiguousarray(np.concatenate(
        [bq_f.reshape(CH, 128).T, bk_f.reshape(CH, 128).T,
         b1_f.reshape(FT, 128).T], axis=1).astype(np.float32))
    # psv holds 32v, psp holds 1024*yWp, pso holds 32*hW2 -> scale biases
    biasR = np.stack([bv_f * WS, bp * WS * WS, b2 * WS]).astype(bf)

    common = {
        "wq": wq_f, "wk": wk_f, "wv": wv_f, "wp": (Wp * WS).astype(f8),
        "w1t": w1_tiled, "w2": w2_st,
        "biasT": biasT, "biasR": biasR,
    }

    in_maps = []
    for core in range(NCORES):
        b = core // 4
        qoff = (core % 4) * QR
        xroll = np.roll(x[b], -qoff, axis=0)
        m = dict(common)
        m["xb"] = xroll.astype(bf)
        m["xr"] = np.ascontiguousarray(x[b][qoff:qoff + QR])
        in_maps.append(m)

    zero_bias_r = not (np.any(bv_f) or np.any(bp) or np.any(b2))
    nc = _get_nc(zero_bias_r)
    _NC_CACHE["last_nc"] = nc
    res = run_bass_kernel_spmd(nc, in_maps, core_ids=list(range(NCORES)))
    _NC_CACHE["last_result"] = res

    outp = np.empty((B, N, C), np.float32)
    for core in range(NCORES):
        b = core // 4
        qoff = (core % 4) * QR
        outp[b, qoff:qoff + QR] = res.results[core]["out"]
    return outp


# revision 13
# speedup vs baseline: 1.0991x; 1.0991x over previous
"""Trainium2 Bass kernel for a dense transformer block (pre-LN, MHA + MLP).

Sharding: 8 cores. Batch (B=2) is split across two 4-core groups; within a
group each core computes the full LayerNorm/K/V for its batch (2048 tokens)
but only its own 512 query rows through attention, proj, and the MLP.
No collectives: the host rolls each core's batch so its query rows are
tokens [0:512], and the host gathers the 8 x [512, 1024] results.

fp8 (e4m3) DoubleRow matmuls everywhere except the S=q@k matmul (bf16) and
the tiny znorm/shift matmuls.  DoubleRow contracts two 128-K tiles per
instruction at 0.5 cycles/output-column (4x bf16 FLOP rate).  Weights are
scaled x32 into fp8 range host-side; descales fold into existing psum-drain
ops (tensor_scalar / activation scale), so they are free.  W1/W2 are split
into fp8 hi+lo pairs (W ~ hi+lo with ~0.3% error) because single-fp8 MLP
weights push the output past the 2e-2 tolerance; attention tolerates plain
fp8 (softmax jitter averages out over 2048 keys).

gamma/beta of both LayerNorms and the q-scale are folded into the weight
matrices host-side (LN(x) @ W == xhat @ (gamma*W) + beta@W).
"""

import numpy as np
import ml_dtypes

import concourse.bass as bass
import concourse.tile as tile
from concourse import bacc, mybir
from concourse.bass_utils import run_bass_kernel_spmd
from concourse.masks import make_identity

F32 = mybir.dt.float32
BF16 = mybir.dt.bfloat16
F8 = mybir.dt.float8e4
AF = mybir.ActivationFunctionType
ALU = mybir.AluOpType
DR = mybir.MatmulPerfMode.DoubleRow

B, N, C, F, H, D = 2, 2048, 1024, 4096, 16, 64
QR = 512            # query rows per core
CH = C // 128       # 8 C-chunks
FT = F // 128       # 32 F-tiles
NT = N // 128       # 16 token tiles
EPS = 1e-6
NCORES = 8
WS = 32.0           # fp8 weight scale
RS = float(1.0 / WS)

_NC_CACHE = {}


def _ln_rstd(nc, stat, mv, eps_sb):
    sd = stat.tile([128, 1], F32, tag="sd", name="sd")
    nc.scalar.activation(out=sd, in_=mv[:, 1:2], func=AF.Sqrt, bias=eps_sb, scale=1.0)
    rstd = stat.tile([128, 1], F32, tag="rstd", name="rstd")
    nc.vector.reciprocal(out=rstd, in_=sd)
    return rstd


def _build_nc(zero_bias_r=False):
    nc = bacc.Bacc("TRN2", target_bir_lowering=False, debug=False,
                   enable_asserts=False, num_devices=NCORES)

    xb = nc.dram_tensor("xb", [N, C], BF16, kind="ExternalInput")
    xr = nc.dram_tensor("xr", [QR, C], F32, kind="ExternalInput")
    wq = nc.dram_tensor("wq", [C, C], F8, kind="ExternalInput")
    wk = nc.dram_tensor("wk", [C, C], F8, kind="ExternalInput")
    wv = nc.dram_tensor("wv", [C, C], F8, kind="ExternalInput")
    wp = nc.dram_tensor("wp", [C, C], F8, kind="ExternalInput")
    w1t = nc.dram_tensor("w1t", [2, FT, 128, CH, 128], F8, kind="ExternalInput")
    w2 = nc.dram_tensor("w2", [2, F, C], F8, kind="ExternalInput")
    biasT = nc.dram_tensor("biasT", [128, CH + CH + FT], F32, kind="ExternalInput")
    biasR = nc.dram_tensor("biasR", [3, C], BF16, kind="ExternalInput")
    out = nc.dram_tensor("out", [QR, C], F32, kind="ExternalOutput")

    with tile.TileContext(nc) as tc, \
         tc.tile_pool(name="consts", bufs=1) as consts, \
         tc.tile_pool(name="wc", bufs=8) as wcp, \
         tc.tile_pool(name="w1p", bufs=6) as w1p, \
         tc.tile_pool(name="big", bufs=1) as big, \
         tc.tile_pool(name="xin", bufs=3) as xin, \
         tc.tile_pool(name="xhp", bufs=3) as xhp, \
         tc.tile_pool(name="stat", bufs=4) as stat, \
         tc.tile_pool(name="expp", bufs=18) as expp, \
         tc.tile_pool(name="outp", bufs=2) as outp, \
         tc.tile_pool(name="zp", bufs=2) as zp, \
         tc.tile_pool(name="ps", bufs=2, space="PSUM") as ps:

        # ---- constants ----
        ident = consts.tile([128, 128], BF16, name="ident")
        make_identity(nc, ident)
        onesP = consts.tile([128, 128], BF16, name="onesP")
        nc.vector.memset(onesP, 1.0)
        shift_sb = consts.tile([128, 128], BF16, name="shift_sb")
        nc.gpsimd.memset(shift_sb, 0.0)
        nc.gpsimd.affine_select(out=shift_sb, in_=shift_sb,
                                compare_op=ALU.not_equal, fill=1.0, base=64,
                                pattern=[[-1, 128]], channel_multiplier=1)
        ones1 = onesP[0:1, :]
        onesF8 = consts.tile([128, 2, 64], F8, name="onesF8")
        nc.vector.memset(onesF8, 1.0)
        eps_sb = consts.tile([128, 1], F32, name="eps_sb")
        nc.vector.memset(eps_sb, EPS)
        biasT_sb = consts.tile([128, CH + CH + FT], F32, name="biasT_sb")
        nc.sync.dma_start(out=biasT_sb, in_=biasT.ap())
        bq_sb = biasT_sb[:, 0:CH]
        bk_sb = biasT_sb[:, CH:2 * CH]
        b1_sb = biasT_sb[:, 2 * CH:2 * CH + FT]
        if not zero_bias_r:
            biasRow = consts.tile([65, C], BF16, name="biasRow")
            nc.sync.dma_start(out=biasRow[::32, :], in_=biasR.ap())
            bv_sb = biasRow[0:1, :]
            bp_sb = biasRow[32:33, :]
            b2_sb = biasRow[64:65, :]
        else:
            bv_sb = bp_sb = b2_sb = None

        def drain_sb(out, in_, eng, scalar1=None, scalar2=None):
            """psum->sbuf drain on DVE ('v') or ACT ('a').
            scalar1: float scale, scalar2: [128,1] bias AP."""
            if eng == 'a':
                if scalar1 is None:
                    nc.scalar.activation(out=out, in_=in_, func=AF.Copy)
                else:
                    nc.scalar.activation(out=out, in_=in_, func=AF.Identity,
                                         bias=scalar2, scale=scalar1)
            else:
                if scalar1 is None:
                    nc.vector.tensor_copy(out=out, in_=in_)
                else:
                    nc.vector.tensor_scalar(out=out, in0=in_, scalar1=scalar1,
                                            scalar2=scalar2,
                                            op0=ALU.mult, op1=ALU.add)

        # ---- LN1 + transpose -> xhatT [128, CH, N] (fp8) ----
        # transposes run in bf16 (fp8 PE-transpose needs strided outputs);
        # 4 chunk-transposes pack into one [128,512] bf16 psum tile and one
        # fp8-converting drain.
        xhatT = big.tile([128, CH, N], F8, tag="xhatT_hT", name="xhatT")

        def emit_ln1(t2):
            xt2 = xin.tile([128, 2, C], BF16, tag="xt", name="xt2")
            nc.sync.dma_start(
                out=xt2, in_=xb.ap()[t2 * 256:(t2 + 1) * 256, :]
                .rearrange("(i p) n -> p i n", p=128))
            for i in range(2):
                t = t2 * 2 + i
                xt = xt2[:, i, :]
                st = stat.tile([128, 2, 6], F32, tag="bns", name="st")
                nc.vector.bn_stats(out=st[:, 0, :], in_=xt[:, 0:512])
                nc.vector.bn_stats(out=st[:, 1, :], in_=xt[:, 512:1024])
                mv = stat.tile([128, 2], F32, tag="mv", name="mv")
                nc.vector.bn_aggr(out=mv, in_=st)
                rstd = _ln_rstd(nc, stat, mv, eps_sb)
                xh = xhp.tile([128, C], BF16, tag="xhat", name="xh", bufs=2)
                # normalize on Pool (sbuf->sbuf; keeps DVE for psum drains)
                nc.gpsimd.tensor_scalar(out=xh, in0=xt, scalar1=mv[:, 0:1],
                                        scalar2=rstd,
                                        op0=ALU.subtract, op1=ALU.mult)
                for half in range(2):
                    pst = ps.tile([128, 4, 128], BF16, tag="mm512", name="pst")
                    for j in range(4):
                        c = half * 4 + j
                        nc.tensor.transpose(pst[:, j, :],
                                            xh[:, c * 128:(c + 1) * 128], ident)
                    drain_sb(xhatT[:, half * 4:half * 4 + 4,
                                   t * 128:(t + 1) * 128],
                             pst, 'a' if half else 'v')

        for t2 in range(2):
            emit_ln1(t2)

        # ---- QKV + attention, two 8-head groups ----
        yT = big.tile([128, H // 2, QR], F8, tag="yT_xh2L", name="yT")

        def dma_w(w, g, name):
            wt = wcp.tile([128, CH, 512], F8, tag="wh", name=name)
            nc.sync.dma_start(
                out=wt, in_=w.ap()[:, g * 512:(g + 1) * 512]
                .rearrange("(c p) n -> p c n", p=128))
            return wt

        def emit_qT(g, wq_h, qT_g, js=None, eng='v'):
            for j in (range(4) if js is None else js):
                psq = ps.tile([128, 512], F32, tag="mm512", name="psq")
                for qh in range(2):
                    for cp in range(4):
                        nc.tensor.matmul(
                            psq[:, qh * 256:(qh + 1) * 256],
                            lhsT=wq_h[:, 2 * cp:2 * cp + 2, j * 128:(j + 1) * 128],
                            rhs=xhatT[:, 2 * cp:2 * cp + 2, qh * 256:(qh + 1) * 256],
                            start=(cp == 0), stop=(cp == 3), perf_mode=DR)
                drain_sb(qT_g[:, j, :], psq, 'a' if (eng == 'x' and j % 2) else eng,
                         RS, bq_sb[:, 4 * g + j:4 * g + j + 1])

        def emit_kT_chunk(g, wk_h, kT_g, j, sl, eng='v'):
            psk = ps.tile([128, 512], F32, tag="mm512", name="psk")
            for qh in range(2):
                off = sl * 512 + qh * 256
                for cp in range(4):
                    nc.tensor.matmul(
                        psk[:, qh * 256:(qh + 1) * 256],
                        lhsT=wk_h[:, 2 * cp:2 * cp + 2, j * 128:(j + 1) * 128],
                        rhs=xhatT[:, 2 * cp:2 * cp + 2, off:off + 256],
                        start=(cp == 0), stop=(cp == 3), perf_mode=DR)
            drain_sb(kT_g[:, j, sl * 512:(sl + 1) * 512], psk, eng,
                     RS, bk_sb[:, 4 * g + j:4 * g + j + 1])

        def emit_v(g, wv_h, vaug, tts=None, eng='v'):
            # vaug holds 32*v in fp8
            for tt in (range(NT) if tts is None else tts):
                psv = ps.tile([128, 512], F32, tag="mm512", name="psv")
                for vh in range(2):
                    for cp in range(4):
                        nc.tensor.matmul(
                            psv[:, vh * 256:(vh + 1) * 256],
                            lhsT=xhatT[:, 2 * cp:2 * cp + 2, tt * 128:(tt + 1) * 128],
                            rhs=wv_h[:, 2 * cp:2 * cp + 2, vh * 256:(vh + 1) * 256],
                            start=(cp == 0),
                            stop=(zero_bias_r and cp == 3), perf_mode=DR)
                if not zero_bias_r:
                    nc.tensor.matmul(psv, lhsT=ones1,
                                     rhs=bv_sb[:, g * 512:(g + 1) * 512],
                                     start=False, stop=True,
                                     skip_group_check=True)
                drain_sb(vaug[:, tt, :, :],
                         psv.rearrange("p (h d) -> p h d", h=8),
                         'a' if (eng == 'x' and tt % 2) else eng)

        def emit_znorm(g, hh, psYZ):
            # psYZ[:,0,:] = 32*sum(e*v); psYZ[:,1,:] = Z (all 64 rows)
            rz = zp.tile([64, 512], BF16, tag="rz", name="rz")
            with nc.allow_low_precision("1/Z in bf16"):
                nc.vector.reciprocal(out=rz, in_=psYZ[:, 1, :])
            pc = (8 * g + hh) // 2
            if hh % 2 == 0:
                nc.vector.tensor_mul(out=yT[0:64, pc, :],
                                     in0=psYZ[:, 0, :], in1=rz)
            else:
                yn = zp.tile([64, 512], BF16, tag="yn", name="yn")
                nc.vector.tensor_mul(out=yn, in0=psYZ[:, 0, :], in1=rz)
                psSh = ps.tile([128, 512], F32, tag="mm512", name="psSh")
                nc.tensor.matmul(psSh, lhsT=shift_sb[0:64, :], rhs=yn,
                                 start=True, stop=True)
                nc.vector.tensor_copy(out=yT[64:128, pc, :],
                                      in_=psSh[64:128, :])

        def emit_S_head(g, hh, kT_g, qT_g, filler):
            jj = hh // 2
            poff = (hh % 2) * 64
            exps = []
            for tp in range(8):
                psS = ps.tile([128, 1024], F32, tag="s1024", name="psS")
                for half in range(2):
                    tt = 2 * tp + half
                    nc.tensor.matmul(
                        psS[:, half * 512:(half + 1) * 512],
                        lhsT=kT_g[poff:poff + 64, jj, tt * 128:(tt + 1) * 128],
                        rhs=qT_g[poff:poff + 64, jj, :],
                        start=True, stop=True)
                expS = expp.tile([128, 2, 512], F8, tag="expS", name="expS")
                nc.scalar.activation(out=expS, in_=psS, func=AF.Exp)
                exps.append(expS)
                if tp == 0 and filler is not None:
                    filler()
            return exps

        def emit_YZ(g, hh, vaug, exps):
            psYZ = ps.tile([64, 2, 512], F32, tag="y", name="psYZ", bufs=1)
            for sel in range(2):            # 0: y = 32*e@v, 1: Z = e@1
                for qh in range(2):
                    qsl = slice(qh * 256, (qh + 1) * 256)
                    for tp in range(8):
                        nc.tensor.matmul(
                            psYZ[:, sel, qsl],
                            lhsT=(vaug[:, 2 * tp:2 * tp + 2, hh, :] if sel == 0
                                  else onesF8),
                            rhs=exps[tp][:, :, qsl],
                            start=(tp == 0), stop=(tp == 7), perf_mode=DR)
            emit_znorm(g, hh, psYZ)

        def emit_attn_group(g, kT_g, qT_g, vaug, filler_for):
            prev = None
            for hh in range(8):
                exps = emit_S_head(g, hh, kT_g, qT_g, filler_for(hh))
                if prev is not None:
                    emit_YZ(g, prev[0], vaug, prev[1])
                prev = (hh, exps)
            emit_YZ(g, prev[0], vaug, prev[1])

        # group 0 QKV, interleaved with the remaining LN1 slices
        wq_h0 = dma_w(wq, 0, "wq_h0")
        wk_h0 = dma_w(wk, 0, "wk_h0")
        wv_h0 = dma_w(wv, 0, "wv_h0")
        qT_g0 = big.tile([128, 4, QR], BF16, tag="qT_xh2T", name="qT_g0", bufs=2)
        emit_qT(0, wq_h0, qT_g0, eng='x')
        kT_g0 = big.tile([128, 4, N], BF16, tag="kT_xr", name="kT_g0", bufs=2)
        vaug0 = big.tile([128, NT, 8, 64], F8, tag="vaug_x2", name="vaug0")
        for sl in range(4):
            for j in range(4):
                emit_kT_chunk(0, wk_h0, kT_g0, j, sl,
                              eng='a' if (j + sl) % 2 else 'v')
            emit_v(0, wv_h0, vaug0, range(4 * sl, 4 * sl + 4), eng='x')
            if sl < 3:
                emit_ln1(2 * (sl + 1))
                emit_ln1(2 * (sl + 1) + 1)

        # group-0 attention: filler worklist = all of qT(g1) plus kT(g1)
        # chunks for head-pairs 0-1; pair-2/3 kT chunks are deferred into
        # group-1's attention.
        wk_h1 = dma_w(wk, 1, "wk_h1")
        wq_h1 = dma_w(wq, 1, "wq_h1")
        kT_g1 = big.tile([128, 4, N], BF16, tag="kT_xr", name="kT_g1", bufs=2)
        qT_g1 = big.tile([128, 4, QR], BF16, tag="qT_xh2T", name="qT_g1", bufs=2)

        units = [lambda j=j: emit_qT(1, wq_h1, qT_g1, [j]) for j in range(4)]
        units += [lambda j=j, sl=sl: emit_kT_chunk(1, wk_h1, kT_g1, j, sl)
                  for j in range(2) for sl in range(4)]

        def g0_filler(hh):
            lo = (hh * len(units)) // 8
            hi = ((hh + 1) * len(units)) // 8

            def fill():
                for u in range(lo, hi):
                    units[u]()
            return fill

        emit_attn_group(0, kT_g0, qT_g0, vaug0, g0_filler)

        # group 1 remaining QKV (v only)
        wv_h1 = dma_w(wv, 1, "wv_h1")
        vaug1 = big.tile([128, NT, 8, 64], F8, tag="vaug_x2", name="vaug1")
        emit_v(1, wv_h1, vaug1)

        # group-1 attention with first-half proj as PE filler.
        # proj: psp = yT(32y) @ wp(32Wp) = 1024*(y@Wp); descale 2^-10 folds
        # into the drain.
        wp_ts = [dma_w(wp, 0, "wp_h0"), dma_w(wp, 1, "wp_h1")]
        proj_part = xhp.tile([128, QR // 128, C], BF16, tag="ppart", name="proj_part",
                             bufs=1)
        PS2 = float(RS * RS)

        def proj_filler(hh):
            os_, qt = hh // 4, hh % 4

            def fill():
                if hh < 4:
                    emit_kT_chunk(1, wk_h1, kT_g1, 2, hh)
                elif hh < 6:
                    emit_kT_chunk(1, wk_h1, kT_g1, 3, 2 * (hh - 4))
                    emit_kT_chunk(1, wk_h1, kT_g1, 3, 2 * (hh - 4) + 1)
                psp = ps.tile([128, 512], F32, tag="mm512", name="psp")
                for ph in range(2):
                    for pp in range(2):
                        nc.tensor.matmul(
                            psp[:, ph * 256:(ph + 1) * 256],
                            lhsT=yT[:, 2 * pp:2 * pp + 2, qt * 128:(qt + 1) * 128],
                            rhs=wp_ts[os_][:, 2 * pp:2 * pp + 2,
                                           ph * 256:(ph + 1) * 256],
                            start=(pp == 0), stop=(pp == 1), perf_mode=DR)
                nc.vector.tensor_scalar(
                    out=proj_part[:, qt, os_ * 512:(os_ + 1) * 512], in0=psp,
                    scalar1=PS2, scalar2=None, op0=ALU.mult)
            return fill

        emit_attn_group(1, kT_g1, qT_g1, vaug1, proj_filler)

        # ---- proj second half + partial + residual -> x2 ----
        xr_sb = big.tile([128, QR // 128, C], F32, tag="kT_xr", name="xr_sb", bufs=2)
        nc.sync.dma_start(out=xr_sb, in_=xr.ap().rearrange("(q p) c -> p q c", p=128))
        x2 = big.tile([128, QR // 128, C], F32, tag="vaug_x2", name="x2")
        for os_ in range(2):
            for qt in range(QR // 128):
                psp = ps.tile([128, 512], F32, tag="mm512", name="psp2")
                for ph in range(2):
                    for pp in range(2, 4):
                        nc.tensor.matmul(
                            psp[:, ph * 256:(ph + 1) * 256],
                            lhsT=yT[:, 2 * pp:2 * pp + 2, qt * 128:(qt + 1) * 128],
                            rhs=wp_ts[os_][:, 2 * pp:2 * pp + 2,
                                           ph * 256:(ph + 1) * 256],
                            start=(pp == 2),
                            stop=(zero_bias_r and pp == 3), perf_mode=DR)
                if not zero_bias_r:
                    nc.tensor.matmul(psp, lhsT=onesP[32:33, :],
                                     rhs=bp_sb[:, os_ * 512:(os_ + 1) * 512],
                                     start=False, stop=True,
                                     skip_group_check=True)
                sl = slice(os_ * 512, (os_ + 1) * 512)
                nc.vector.scalar_tensor_tensor(
                    out=x2[:, qt, sl], in0=psp, scalar=PS2,
                    in1=proj_part[:, qt, sl], op0=ALU.mult, op1=ALU.add)
                nc.vector.tensor_add(out=x2[:, qt, sl], in0=x2[:, qt, sl],
                                     in1=xr_sb[:, qt, sl])

        # ---- LN2 + transpose -> xhat2T hi/lo [128, CH, QR] (fp8) ----
        # xh2T is the fp8 rounding of xhat2; xh2L holds the rounding residual
        # (also fp8, unscaled: residuals live in fp8's subnormal range), so
        # MLP1 can contract (hi + lo) @ W1_hi for ~0.1% effective input
        # precision.
        xh2T = big.tile([128, CH, QR], F8, tag="qT_xh2T", name="xh2T", bufs=2)
        xh2L = big.tile([128, CH, QR], F8, tag="yT_xh2L", name="xh2L")
        for qt in range(QR // 128):
            st2 = stat.tile([128, 2, 6], F32, tag="bns", name="st2")
            nc.vector.bn_stats(out=st2[:, 0, :], in_=x2[:, qt, 0:512])
            nc.vector.bn_stats(out=st2[:, 1, :], in_=x2[:, qt, 512:1024])
            mv2 = stat.tile([128, 2], F32, tag="mv", name="mv2")
            nc.vector.bn_aggr(out=mv2, in_=st2)
            rstd2 = _ln_rstd(nc, stat, mv2, eps_sb)
            xh2 = xhp.tile([128, C], BF16, tag="xhat", name="xh2", bufs=2)
            nc.gpsimd.tensor_scalar(out=xh2, in0=x2[:, qt, :], scalar1=mv2[:, 0:1],
                                    scalar2=rstd2, op0=ALU.subtract, op1=ALU.mult)
            for half in range(2):
                pst2 = ps.tile([128, 4, 128], BF16, tag="mm512", name="pst2")
                for j in range(4):
                    c = half * 4 + j
                    nc.tensor.transpose(pst2[:, j, :],
                                        xh2[:, c * 128:(c + 1) * 128], ident)
                hsl = slice(half * 4, half * 4 + 4)
                qsl = slice(qt * 128, (qt + 1) * 128)
                nc.vector.tensor_copy(out=xh2T[:, hsl, qsl], in_=pst2)
                nc.vector.tensor_sub(out=xh2L[:, hsl, qsl], in0=pst2,
                                     in1=xh2T[:, hsl, qsl])

        # ---- MLP1: hT [128, FT, QR] fp8, fused exact Gelu + bias ----
        # psh = 32*(xn2 @ W1'): gelu(psh * 2^-5 + b1)
        hT = big.tile([128, FT, QR], F8, tag="xhatT_hT", name="hT")
        for ft in range(FT):
            w1hi = w1p.tile([128, CH, 128], F8, tag="w1", name="w1hi")
            nc.sync.dma_start(out=w1hi, in_=w1t.ap()[0, ft])
            w1lo = w1p.tile([128, CH, 128], F8, tag="w1", name="w1lo")
            nc.sync.dma_start(out=w1lo, in_=w1t.ap()[1, ft])
            psh = ps.tile([128, 512], F32, tag="mm512", name="psh")
            for qh in range(2):
                qsl = slice(qh * 256, (qh + 1) * 256)
                for cp in range(4):
                    nc.tensor.matmul(
                        psh[:, qsl], lhsT=w1hi[:, 2 * cp:2 * cp + 2, :],
                        rhs=xh2T[:, 2 * cp:2 * cp + 2, qsl],
                        start=(cp == 0), stop=False, perf_mode=DR)
                for cp in range(4):
                    nc.tensor.matmul(
                        psh[:, qsl], lhsT=w1lo[:, 2 * cp:2 * cp + 2, :],
                        rhs=xh2T[:, 2 * cp:2 * cp + 2, qsl],
                        start=False, stop=False, perf_mode=DR)
                for cp in range(4):
                    nc.tensor.matmul(
                        psh[:, qsl], lhsT=w1hi[:, 2 * cp:2 * cp + 2, :],
                        rhs=xh2L[:, 2 * cp:2 * cp + 2, qsl],
                        start=False, stop=(cp == 3), perf_mode=DR)
            nc.scalar.activation(out=hT[:, ft, :], in_=psh, func=AF.Gelu,
                                 bias=b1_sb[:, ft:ft + 1], scale=RS)

        # ---- MLP2 + residual -> out;  pso = 32*(h @ W2') ----
        for os_ in range(2):
            w2_hi, w2_lo = [], []
            for i in range(4):
                for dst, hl in ((w2_hi, 0), (w2_lo, 1)):
                    w2t = wcp.tile([128, CH, 512], F8, tag="wh", name="w2t")
                    nc.sync.dma_start(
                        out=w2t, in_=w2.ap()[hl, i * 1024:(i + 1) * 1024,
                                             os_ * 512:(os_ + 1) * 512]
                        .rearrange("(c p) n -> p c n", p=128))
                    dst.append(w2t)
            for qt in range(QR // 128):
                pso = ps.tile([128, 512], F32, tag="mm512", name="pso")
                for ph in range(2):
                    for src, first, last in ((w2_hi, True, False),
                                             (w2_lo, False, True)):
                        for fp in range(16):
                            nc.tensor.matmul(
                                pso[:, ph * 256:(ph + 1) * 256],
                                lhsT=hT[:, 2 * fp:2 * fp + 2,
                                        qt * 128:(qt + 1) * 128],
                                rhs=src[fp // 4][:, 2 * (fp % 4):2 * (fp % 4) + 2,
                                                 ph * 256:(ph + 1) * 256],
                                start=(first and fp == 0),
                                stop=(last and fp == 15 and zero_bias_r),
                                perf_mode=DR)
                if not zero_bias_r:
                    nc.tensor.matmul(pso, lhsT=onesP[64:65, :],
                                     rhs=b2_sb[:, os_ * 512:(os_ + 1) * 512],
                                     start=False, stop=True,
                                     skip_group_check=True)
                ot = outp.tile([128, 512], F32, tag="ot", name="ot")
                nc.vector.scalar_tensor_tensor(
                    out=ot, in0=pso, scalar=RS,
                    in1=x2[:, qt, os_ * 512:(os_ + 1) * 512],
                    op0=ALU.mult, op1=ALU.add)
                nc.sync.dma_start(out=out.ap()[qt * 128:(qt + 1) * 128,
                                               os_ * 512:(os_ + 1) * 512], in_=ot)

    nc.finalize()
    return nc


def _get_nc(zero_bias_r=False):
    key = ("nc", zero_bias_r)
    if key not in _NC_CACHE:
        _NC_CACHE[key] = _build_nc(zero_bias_r)
    return _NC_CACHE[key]


def kernel(x, Wq, Wk, Wv, Wp, bp, W1, b1, W2, b2, gamma1, beta1, gamma2, beta2):
    bf = ml_dtypes.bfloat16
    f8 = ml_dtypes.float8_e4m3
    x = np.asarray(x, np.float32)
    Wq = np.asarray(Wq, np.float32)
    Wk = np.asarray(Wk, np.float32)
    Wv = np.asarray(Wv, np.float32)
    Wp = np.asarray(Wp, np.float32)
    bp = np.asarray(bp, np.float32)
    W1 = np.asarray(W1, np.float32)
    b1 = np.asarray(b1, np.float32)
    W2 = np.asarray(W2, np.float32)
    b2 = np.asarray(b2, np.float32)
    gamma1 = np.asarray(gamma1, np.float32)
    beta1 = np.asarray(beta1, np.float32)
    gamma2 = np.asarray(gamma2, np.float32)
    beta2 = np.asarray(beta2, np.float32)

    scale = np.float32(D ** -0.5)
    wq_f = ((gamma1[:, None] * Wq) * (scale * WS)).astype(f8)
    bq_f = (beta1 @ Wq) * scale
    wk_f = ((gamma1[:, None] * Wk) * WS).astype(f8)
    bk_f = beta1 @ Wk
    wv_f = ((gamma1[:, None] * Wv) * WS).astype(f8)
    bv_f = beta1 @ Wv
    w1_f = (gamma2[:, None] * W1) * WS
    b1_f = b1 + beta2 @ W1

    def hilo(Ws):
        hi = Ws.astype(f8)
        lo = (Ws - hi.astype(np.float32)).astype(f8)
        return hi, lo

    w1_hi, w1_lo = hilo(w1_f)
    w1_tiled = np.stack([
        np.ascontiguousarray(
            w.reshape(CH, 128, FT, 128).transpose(2, 1, 0, 3))
        for w in (w1_hi, w1_lo)])
    w2_hi, w2_lo = hilo(W2 * WS)
    w2_st = np.stack([w2_hi, w2_lo])
    biasT = np.ascontiguousarray(np.concatenate(
        [bq_f.reshape(CH, 128).T, bk_f.reshape(CH, 128).T,
         b1_f.reshape(FT, 128).T], axis=1).astype(np.float32))
    # psv holds 32v, psp holds 1024*yWp, pso holds 32*hW2 -> scale biases
    biasR = np.stack([bv_f * WS, bp * WS * WS, b2 * WS]).astype(bf)

    common = {
        "wq": wq_f, "wk": wk_f, "wv": wv_f, "wp": (Wp * WS).astype(f8),
        "w1t": w1_tiled, "w2": w2_st,
        "biasT": biasT, "biasR": biasR,
    }

    in_maps = []
    for core in range(NCORES):
        b = core // 4
        qoff = (core % 4) * QR
        xroll = np.roll(x[b], -qoff, axis=0)
        m = dict(common)
        m["xb"] = xroll.astype(bf)
        m["xr"] = np.ascontiguousarray(x[b][qoff:qoff + QR])
        in_maps.append(m)

    zero_bias_r = not (np.any(bv_f) or np.any(bp) or np.any(b2))
    nc = _get_nc(zero_bias_r)
    _NC_CACHE["last_nc"] = nc
    res = run_bass_kernel_spmd(nc, in_maps, core_ids=list(range(NCORES)))
    _NC_CACHE["last_result"] = res

    outp = np.empty((B, N, C), np.float32)
    for core in range(NCORES):
        b = core // 4
        qoff = (core % 4) * QR
        outp[b, qoff:qoff + QR] = res.results[core]["out"]
    return outp
